# revision 51
# baseline (speedup 1.0000x reference)
"""EnhancedCondConv2d on 8 Trainium2 NeuronCores.

Strategy (data-parallel over batch, 4 samples per core):
  - routing SE MLP + softmax computed on-device from fp32 x
  - per-sample expert weight combination on the vector engine (bf16)
  - channel attention computed BEFORE the conv via linearity of mean
    pooling (windowed sums from row/col sums), folded into the weights
  - 3x3 conv as 9 shift-matmuls accumulating in PSUM (bf16 operands,
    fp32 accumulation), PSUM chunks DMA'd straight to DRAM output
"""

import os
import sys

import numpy as np

sys.path.insert(0, "/opt/trn_rl_repo")

import ml_dtypes

import concourse.bass as bass
import concourse.mybir as mybir
import concourse.tile as tile
B, CI, CO, H, W, E, R, K = 32, 128, 256, 64, 64, 8, 16, 3
NCORES = 8
BL = B // NCORES          # samples per core
HP, WP = 66, 68           # padded x: rows 1..64 and cols 2..65 hold data
NPIX = H * W
F32 = mybir.dt.float32
BF16 = mybir.dt.bfloat16
AF = mybir.ActivationFunctionType
ALU = mybir.AluOpType
AX = mybir.AxisListType



def _build_nc(repeat=1, variant="full", loop_n=0):
    nc = bass.Bass()

    x_d = nc.declare_dram_parameter("xloc", [BL, CI, H, W], F32, False)
    e_d = nc.declare_dram_parameter("experts_t", [CI, E, 9, CO], BF16, False)
    rw1t_d = nc.declare_dram_parameter("rw1t", [CI, E], F32, False)
    rb1_d = nc.declare_dram_parameter("rb1", [CI // R, 1], F32, False)
    rw2t_d = nc.declare_dram_parameter("rw2t", [CI // R, CI], F32, False)
    rb2_d = nc.declare_dram_parameter("rb2", [CI, 1], F32, False)
    rw3t_d = nc.declare_dram_parameter("rw3t", [CI, E], F32, False)
    rb3_d = nc.declare_dram_parameter("rb3", [E, 1], F32, False)
    aw1t_d = nc.declare_dram_parameter("aw1t", [CI, 2 * (CO // R)], F32, False)
    ab1_d = nc.declare_dram_parameter("ab1", [CO // R, 1], F32, False)
    aw2t_d = nc.declare_dram_parameter("aw2t", [CO // R, CO], F32, False)
    ab2p_d = nc.declare_dram_parameter("ab2p", [128, 2], F32, False)
    onesc_d = nc.declare_dram_parameter("onesc", [CI, 1], F32, False)
    id8_d = nc.declare_dram_parameter("id8", [E, E], F32, False)
    ones8_d = nc.declare_dram_parameter("ones8", [E, CI], F32, False)
    out_d = nc.declare_dram_parameter("out", [BL, CO, H, W], F32, True)

    with (
        tile.TileContext(nc) as tc,
        tc.tile_pool(name="const", bufs=1) as constp,
        tc.tile_pool(name="xload", bufs=2) as xloadp,
        tc.tile_pool(name="wt", bufs=2) as wtp,
        tc.tile_pool(name="small", bufs=3) as smallp,
        tc.tile_pool(name="ostage", bufs=8) as ostagep,
        tc.tile_pool(name="pconv", bufs=5, space="PSUM") as pconv,
        tc.tile_pool(name="psmall", bufs=3, space="PSUM") as psmall,
    ):
        # ---- constants ----
        experts_sb = constp.tile([CI, E, 9, CO], BF16)
        for e in range(E):
            nc.gpsimd.dma_start(experts_sb[:, e], e_d[:, e])
        rw1t_sb = constp.tile([CI, E], F32)
        nc.sync.dma_start(rw1t_sb[:], rw1t_d[:])
        rw2t_sb = constp.tile([CI // R, CI], F32)
        nc.sync.dma_start(rw2t_sb[:], rw2t_d[:])
        rw3t_sb = constp.tile([CI, E], F32)
        nc.sync.dma_start(rw3t_sb[:], rw3t_d[:])
        rb1_sb = constp.tile([CI // R, 1], F32)
        nc.sync.dma_start(rb1_sb[:], rb1_d[:])
        rb2_sb = constp.tile([CI, 1], F32)
        nc.sync.dma_start(rb2_sb[:], rb2_d[:])
        rb3_sb = constp.tile([E, 1], F32)
        nc.sync.dma_start(rb3_sb[:], rb3_d[:])
        aw1t_sb = constp.tile([CI, 2, CO // R], F32)
        nc.sync.dma_start(aw1t_sb[:], aw1t_d[:].rearrange("p (h m) -> p h m", h=2))
        ab1_sb = constp.tile([CO // R, 1], F32)
        nc.sync.dma_start(ab1_sb[:], ab1_d[:])
        aw2t_sb = constp.tile([CO // R, 2, 128], F32)
        nc.sync.dma_start(aw2t_sb[:], aw2t_d[:].rearrange("k (h m) -> k h m", h=2))
        ab2p_sb = constp.tile([128, 2], F32)
        nc.sync.dma_start(ab2p_sb[:], ab2p_d[:])
        onesc_sb = constp.tile([CI, 1], F32)
        nc.sync.dma_start(onesc_sb[:], onesc_d[:])
        id8_sb = constp.tile([E, E], F32)
        nc.sync.dma_start(id8_sb[:], id8_d[:])
        ones8_sb = constp.tile([E, CI], F32)
        nc.sync.dma_start(ones8_sb[:], ones8_d[:])

        # per-sample persistent padded-x and weight tiles; the xpad zero
        # border is written once and survives (per-sample copies only touch
        # the interior)
        xpads, waccs, totals, caps = [], [], [], []
        for i in range(BL):
            t = constp.tile([CI, HP, WP], BF16, name=f"xpad{i}", tag=f"xpad{i}")
            nc.gpsimd.memset(t[:], 0.0)
            xpads.append(t)
            w = [
                constp.tile([CI, 3, CO], BF16, name=f"wacc{i}g{g}", tag=f"wacc{i}g{g}")
                for g in range(3)
            ]
            tt = constp.tile([CI, 1], F32, name=f"total{i}", tag=f"total{i}")
            totals.append(tt)
            cp = constp.tile([128, 2], F32, name=f"cap{i}", tag=f"cap{i}")
            caps.append(cp)
            if variant == "bonly":
                for wg in w:
                    nc.gpsimd.memset(wg[:], 0.5)
            waccs.append(w)

        def stage_f(b):
            """load + routing softmax + expert combine into waccs[b]."""
            xpad, wacc = xpads[b], waccs[b]
            x32 = xloadp.tile([CI, H, W], F32, name="x32", tag="x32")
            for q in range(4):
                nc.sync.dma_start(
                    x32[:, q * (H // 4) : (q + 1) * (H // 4)],
                    x_d[b, :, q * (H // 4) : (q + 1) * (H // 4)],
                )
            # routing total on DVE straight from fp32 x, in parallel with the
            # ACT pad-converts (keeps the MLP off the convert critical path)
            tot4 = smallp.tile([CI, 4], F32, name="tot4", tag="tot4")
            for q in range(4):
                nc.vector.tensor_reduce(
                    tot4[:, q : q + 1],
                    x32[:, q * (H // 4) : (q + 1) * (H // 4)],
                    axis=AX.XY,
                    op=ALU.add,
                )
            total = totals[b]
            nc.vector.tensor_reduce(total[:], tot4[:], axis=AX.X, op=ALU.add)
            nc.scalar.activation(
                xpad[:, 1 : H // 2 + 1, 2 : W + 2], x32[:, 0 : H // 2], AF.Copy
            )
            nc.scalar.activation(
                xpad[:, H // 2 + 1 : H + 1, 2 : W + 2], x32[:, H // 2 : H], AF.Copy
            )
            # routing MLP -> expert weights r[e], broadcast to [CI,1]
            ph1 = psmall.tile([CI // R, 1], F32, name="psm", tag="psm")
            nc.tensor.matmul(ph1[:], lhsT=rw1t_sb[:], rhs=total[:], start=True, stop=True)
            h1 = smallp.tile([CI // R, 1], F32, name="h1", tag="h1")
            nc.scalar.activation(h1[:], ph1[:], AF.Relu, bias=rb1_sb[:, 0:1], scale=1.0 / NPIX)

            ps = psmall.tile([CI, 1], F32, name="psm", tag="psm")
            nc.tensor.matmul(ps[:], lhsT=rw2t_sb[:], rhs=h1[:], start=True, stop=True)
            sg = smallp.tile([CI, 1], F32, name="sg", tag="sg")
            nc.scalar.activation(sg[:], ps[:], AF.Sigmoid, bias=rb2_sb[:, 0:1])

            pl = psmall.tile([E, 1], F32, name="psm", tag="psm")
            nc.tensor.matmul(pl[:], lhsT=rw3t_sb[:], rhs=sg[:], start=True, stop=True)
            expv = smallp.tile([E, 1], F32, name="expv", tag="expv")
            nc.scalar.activation(expv[:], pl[:], AF.Exp, bias=rb3_sb[:, 0:1])

            # one matmul broadcasts exp[e] (cols 0..7) and their sum (col 8)
            # across all 128 partitions: ones8^T @ [diag(exp) | exp]
            diag9 = smallp.tile([E, E + 1], F32, name="diag9", tag="diag9")
            nc.vector.tensor_scalar_mul(diag9[:, 0:E], id8_sb[:], expv[:, 0:1])
            nc.vector.tensor_copy(out=diag9[:, E : E + 1], in_=expv[:])
            pbc = psmall.tile([CI, E + 1], F32, name="psm", tag="psm")
            nc.tensor.matmul(pbc[:], lhsT=ones8_sb[:], rhs=diag9[:], start=True, stop=True)
            rinv = smallp.tile([CI, 1], F32, name="rinv", tag="rinv")
            nc.vector.reciprocal(rinv[:], pbc[:, E : E + 1])
            rcol = smallp.tile([CI, E], F32, name="rcol", tag="rcol")
            nc.vector.tensor_scalar_mul(rcol[:], pbc[:, 0:E], rinv[:, 0:1])

            # combine experts: w[ci, dydx, co] = sum_e r[e] * expert_e,
            # in three tap-group tiles so the conv's first taps start after
            # one third of the combine instead of all of it
            for g in range(3):
                wg = wacc[g]
                wtmp = wtp.tile([CI, 3, CO], BF16, name="wtmp", tag="wtmp")
                nc.vector.tensor_scalar_mul(
                    wg[:], experts_sb[:, 0, 3 * g : 3 * g + 3], rcol[:, 0:1]
                )
                for e in range(1, E):
                    nc.vector.tensor_scalar_mul(
                        wtmp[:], experts_sb[:, e, 3 * g : 3 * g + 3], rcol[:, e : e + 1]
                    )
                    nc.vector.tensor_add(wg[:], wg[:], wtmp[:])

        def stage_g(b):
            """windowed sums -> exact mean-pooled conv output -> channel
            attention, folded into waccs[b]."""
            xpad, wacc = xpads[b], waccs[b]
            total = totals[b]
            edge = smallp.tile([CI, 4], F32, name="edge", tag="edge")
            nc.vector.tensor_reduce(edge[:, 0:1], xpad[:, 1, :], axis=AX.X, op=ALU.add)
            nc.vector.tensor_reduce(edge[:, 1:2], xpad[:, 64, :], axis=AX.X, op=ALU.add)
            nc.vector.tensor_reduce(edge[:, 2:3], xpad[:, :, 2], axis=AX.X, op=ALU.add)
            nc.vector.tensor_reduce(edge[:, 3:4], xpad[:, :, 65], axis=AX.X, op=ALU.add)

            # windowed sums S[ci, dydx] from total/edge sums
            Sf = smallp.tile([CI, 9], F32, name="Sf", tag="Sf")
            nc.vector.tensor_copy(out=Sf[:], in_=total[:, 0:1].to_broadcast([CI, 9]))
            nc.vector.tensor_sub(
                Sf[:, 0:3], Sf[:, 0:3], edge[:, 1:2].to_broadcast([CI, 3])
            )
            nc.vector.tensor_sub(
                Sf[:, 6:9], Sf[:, 6:9], edge[:, 0:1].to_broadcast([CI, 3])
            )
            for dy in range(3):
                nc.vector.tensor_sub(
                    Sf[:, dy * 3 : dy * 3 + 1], Sf[:, dy * 3 : dy * 3 + 1], edge[:, 3:4]
                )
                nc.vector.tensor_sub(
                    Sf[:, dy * 3 + 2 : dy * 3 + 3], Sf[:, dy * 3 + 2 : dy * 3 + 3], edge[:, 2:3]
                )
            nc.vector.tensor_add(Sf[:, 0:1], Sf[:, 0:1], xpad[:, 64, 65:66])
            nc.vector.tensor_add(Sf[:, 2:3], Sf[:, 2:3], xpad[:, 64, 2:3])
            nc.vector.tensor_add(Sf[:, 6:7], Sf[:, 6:7], xpad[:, 1, 65:66])
            nc.vector.tensor_add(Sf[:, 8:9], Sf[:, 8:9], xpad[:, 1, 2:3])
            Sbf = smallp.tile([CI, 9], BF16, name="Sbf", tag="Sbf")
            nc.vector.tensor_copy(out=Sbf[:], in_=Sf[:])

            # mean-pooled conv output (exact, via linearity)
            ppool = psmall.tile([128, 2], F32, name="psm", tag="psm")
            for h in range(2):
                for j in range(9):
                    nc.tensor.matmul(
                        ppool[:, h : h + 1],
                        lhsT=wacc[j // 3][:, j % 3, h * 128 : (h + 1) * 128],
                        rhs=Sbf[:, j : j + 1],
                        start=(j == 0),
                        stop=(j == 8),
                    )
            pool_sb = smallp.tile([128, 2], F32, name="pool_sb", tag="pool_sb")
            nc.scalar.copy(pool_sb[:], ppool[:])

            ph2 = psmall.tile([CO // R, 1], F32, name="psm", tag="psm")
            nc.tensor.matmul(ph2[:], lhsT=aw1t_sb[:, 0], rhs=pool_sb[:, 0:1], start=True, stop=False)
            nc.tensor.matmul(ph2[:], lhsT=aw1t_sb[:, 1], rhs=pool_sb[:, 1:2], start=False, stop=True)
            h2 = smallp.tile([CO // R, 1], F32, name="h2", tag="h2")
            nc.scalar.activation(h2[:], ph2[:], AF.Relu, bias=ab1_sb[:, 0:1], scale=1.0 / NPIX)

            # ca in co-partition layout [128, 2]; applied as a per-partition
            # scale when the conv PSUM chunks drain (no weight fold needed)
            pca = psmall.tile([128, 2], F32, name="psm", tag="psm")
            for h in range(2):
                nc.tensor.matmul(
                    pca[:, h : h + 1], lhsT=aw2t_sb[:, h], rhs=h2[:], start=True, stop=True
                )
            cap = caps[b]
            for h in range(2):
                nc.scalar.activation(
                    cap[:, h : h + 1], pca[:, h : h + 1], AF.Sigmoid,
                    bias=ab2p_sb[:, h : h + 1],
                )

        def stage_b(b, h):
            """the conv, one co-half: 8 chunks x 9 accumulating taps; the
            channel-attention scale is applied during the PSUM drain."""
            xpad, wacc, cap = xpads[b], waccs[b], caps[b]
            if True:
                for c in range(8):
                    y0 = c * 8
                    pt = pconv.tile([128, 512], F32, tag="cv", name="cv")
                    for j in range(9):
                        dy, dx = j // 3, j % 3
                        nc.tensor.matmul(
                            pt[:],
                            lhsT=wacc[j // 3][:, j % 3, h * 128 : (h + 1) * 128],
                            rhs=xpad[:, y0 + dy : y0 + dy + 8, dx + 1 : dx + 65],
                            start=(j == 0),
                            stop=(j == 8),
                        )
                    stage = ostagep.tile([128, 512], F32, tag="ostage", name="ostage")
                    nc.scalar.activation(
                        stage[:], pt[:], AF.Copy, scale=cap[:, h : h + 1]
                    )
                    nc.sync.dma_start(
                        out_d[b, h * 128 : (h + 1) * 128, y0 : y0 + 8, :],
                        stage[:],
                    )

        # software pipeline: stage A runs two samples ahead of stage B so the
        # vector-engine work of sample b+1/b+2 hides under sample b's conv
        import contextlib
        loop_cm = tc.For_i(0, loop_n, 1) if loop_n > 0 else contextlib.nullcontext()
        with loop_cm:
            for _rep in range(repeat):
                if variant == "aonly":
                    for b in range(BL):
                        stage_f(b)
                        stage_g(b)
                elif variant == "bonly":
                    for b in range(BL):
                        stage_b(b, 0)
                        stage_b(b, 1)
                else:
                    stage_f(0)
                    stage_g(0)
                    stage_f(1)
                    stage_b(0, 0)
                    stage_b(0, 1)
                    stage_g(1)
                    stage_f(2)
                    stage_b(1, 0)
                    stage_b(1, 1)
                    stage_g(2)
                    stage_f(3)
                    stage_b(2, 0)
                    stage_b(2, 1)
                    stage_g(3)
                    stage_b(3, 0)
                    stage_b(3, 1)
    return nc


def _split_multi_waits(nc):
    """The walrus build in this container only encodes one sync-wait per
    instruction. Split extra waits into standalone EventSemaphore ops on the
    same engine immediately before the instruction (identical blocking
    semantics for in-order sequencers)."""
    ctr = 0
    for f in nc.m.functions:
        for bb in f.blocks:
            out = []
            for inst in bb.instructions:
                si = inst.sync_info
                if si is not None and si.on_wait and len(si.on_wait) > 1:
                    waits = list(si.on_wait)
                    for wt in waits[:-1]:
                        ev = mybir.InstEventSemaphore(name=f"evsplit-{ctr}", ins=[], outs=[])
                        ctr += 1
                        ev.engine = inst.engine
                        ev.sync_info = mybir.SyncInfo(on_wait=[wt], on_update=[])
                        out.append(ev)
                    si.on_wait = [waits[-1]]
                out.append(inst)
            bb.instructions = out


_NC_CACHE_R = {}


def _get_nc(repeat=1, variant="full", loop_n=0):
    global _NC_CACHE_R
    key = (repeat, variant, loop_n)
    if key not in _NC_CACHE_R:
        nc = _build_nc(repeat, variant, loop_n)
        _split_multi_waits(nc)
        _NC_CACHE_R[key] = nc
    return _NC_CACHE_R[key]


def _prep_maps(x, experts, rw1, rb1, rw2, rb2, rw3, rb3, aw1, ab1, aw2, ab2):
    f32 = np.float32
    experts_t = np.ascontiguousarray(
        np.transpose(experts.astype(f32), (2, 0, 3, 4, 1)).reshape(CI, E, 9, CO)
    ).astype(ml_dtypes.bfloat16)
    aw1t = np.ascontiguousarray(
        aw1.astype(f32).T.reshape(2, 128, CO // R).transpose(1, 0, 2).reshape(CI, 2 * (CO // R))
    )

    shared = {
        "experts_t": experts_t,
        "rw1t": np.ascontiguousarray(rw1.astype(f32).T),
        "rb1": np.ascontiguousarray(rb1.astype(f32).reshape(-1, 1)),
        "rw2t": np.ascontiguousarray(rw2.astype(f32).T),
        "rb2": np.ascontiguousarray(rb2.astype(f32).reshape(-1, 1)),
        "rw3t": np.ascontiguousarray(rw3.astype(f32).T),
        "rb3": np.ascontiguousarray(rb3.astype(f32).reshape(-1, 1)),
        "aw1t": aw1t,
        "ab1": np.ascontiguousarray(ab1.astype(f32).reshape(-1, 1)),
        "aw2t": np.ascontiguousarray(aw2.astype(f32).T),
        "ab2p": np.ascontiguousarray(ab2.astype(f32).reshape(2, 128).T),
        "onesc": np.ones((CI, 1), f32),
        "id8": np.eye(E, dtype=f32),
        "ones8": np.ones((E, CI), f32),
    }
    in_maps = []
    for c in range(NCORES):
        m = dict(shared)
        m["xloc"] = np.ascontiguousarray(x[c * BL : (c + 1) * BL].astype(f32))
        in_maps.append(m)
    return in_maps


_COMPILED = {}


def _get_compiled(repeat=1, variant="full", loop_n=0):
    """Build the Bass program once and wrap it in a cached shard_map-jitted
    callable over the 8 NeuronCores (mirrors bass2jax.run_bass_via_pjrt but
    keeps the jitted function alive so repeat calls skip recompilation)."""
    global _COMPILED
    key = (repeat, variant, loop_n)
    if key in _COMPILED:
        return _COMPILED[key]

    import jax
    from jax.experimental.shard_map import shard_map
    from jax.sharding import Mesh, PartitionSpec

    from concourse import bass2jax, mybir as _mybir

    nc = _get_nc(repeat, variant, loop_n)
    bass2jax.install_neuronx_cc_hook()

    partition_name = nc.partition_id_tensor.name if nc.partition_id_tensor else None
    in_names, out_names, out_avals, zero_shapes = [], [], [], []
    for alloc in nc.m.functions[0].allocations:
        if not isinstance(alloc, _mybir.MemoryLocationSet):
            continue
        name = alloc.memorylocations[0].name
        if alloc.kind == "ExternalInput":
            if name != partition_name:
                in_names.append(name)
        elif alloc.kind == "ExternalOutput":
            out_names.append(name)
            shape = tuple(alloc.tensor_shape)
            dtype = _mybir.dt.np(alloc.dtype)
            out_avals.append(jax.core.ShapedArray(shape, dtype))
            zero_shapes.append((shape, dtype))
    n_params = len(in_names)
    all_names = in_names + out_names
    if partition_name is not None:
        all_names = all_names + [partition_name]
    donate = tuple(range(n_params, n_params + len(out_names)))

    def _body(*args):
        operands = list(args)
        if partition_name is not None:
            operands.append(bass2jax.partition_id_tensor())
        outs = bass2jax._bass_exec_p.bind(
            *operands,
            out_avals=tuple(out_avals),
            in_names=tuple(all_names),
            out_names=tuple(out_names),
            lowering_input_output_aliases=(),
            sim_require_finite=True,
            sim_require_nnan=True,
            nc=nc,
        )
        return tuple(outs)

    devices = jax.devices()[:NCORES]
    mesh = Mesh(np.asarray(devices), ("core",))
    specs = (PartitionSpec("core"),) * (n_params + len(out_names))
    sharded = jax.jit(
        shard_map(
            _body, mesh=mesh, in_specs=specs,
            out_specs=(PartitionSpec("core"),) * len(out_names),
            check_rep=False,
        ),
        donate_argnums=donate,
        keep_unused=True,
    )
    from jax.sharding import NamedSharding
    import jax.numpy as jnp

    sh = NamedSharding(mesh, PartitionSpec("core"))
    zmaker = jax.jit(
        lambda: tuple(
            jnp.zeros((NCORES * s[0], *s[1:]), d) for s, d in zero_shapes
        ),
        out_shardings=tuple(sh for _ in zero_shapes),
    )
    _COMPILED[key] = (sharded, in_names, out_names, zero_shapes, mesh, zmaker)
    return _COMPILED[key]


def _concat_inputs(in_maps, in_names):
    return [
        np.concatenate([m[name] for m in in_maps], axis=0) for name in in_names
    ]


_DEV_CACHE = {}


def _to_device(arrs, mesh):
    """Cache device-resident input buffers keyed by content hash (inputs are
    not donated, so reuse across calls is safe)."""
    import hashlib

    import jax
    from jax.sharding import NamedSharding, PartitionSpec

    sh = NamedSharding(mesh, PartitionSpec("core"))
    out = []
    for a in arrs:
        key = (a.shape, str(a.dtype), hashlib.md5(a.tobytes()).hexdigest())
        buf = _DEV_CACHE.get(key)
        if buf is None:
            buf = jax.device_put(a, sh)
            _DEV_CACHE[key] = buf
        out.append(buf)
    if len(_DEV_CACHE) > 64:
        _DEV_CACHE.clear()
    return out


def kernel(**inputs):
    inputs = {k: np.asarray(v) for k, v in inputs.items()}
    sharded, in_names, out_names, zero_shapes, mesh, zmaker = _get_compiled()
    in_maps = _prep_maps(
        inputs["x"], inputs["experts"],
        inputs["rw1"], inputs["rb1"], inputs["rw2"], inputs["rb2"],
        inputs["rw3"], inputs["rb3"], inputs["aw1"], inputs["ab1"],
        inputs["aw2"], inputs["ab2"],
    )
    concat_in = _to_device(_concat_inputs(in_maps, in_names), mesh)
    zeros = zmaker()
    out_arrs = sharded(*concat_in, *zeros)
    out = np.asarray(out_arrs[out_names.index("out")])
    return np.ascontiguousarray(out).astype(np.float32)


def _chain_time(inputs, repeat, iters):
    import time

    import jax
    from jax.sharding import NamedSharding, PartitionSpec

    variant = os.environ.get("KERNEL_VARIANT", "full")
    loop_n = int(os.environ.get("KERNEL_LOOP", "0"))
    sharded, in_names, out_names, zero_shapes, mesh, zmaker = _get_compiled(
        repeat, variant, loop_n
    )
    in_maps = _prep_maps(
        inputs["x"], inputs["experts"],
        inputs["rw1"], inputs["rb1"], inputs["rw2"], inputs["rb2"],
        inputs["rw3"], inputs["rb3"], inputs["aw1"], inputs["ab1"],
        inputs["aw2"], inputs["ab2"],
    )
    concat_in = _concat_inputs(in_maps, in_names)
    sh = NamedSharding(mesh, PartitionSpec("core"))
    dev_in = [jax.device_put(a, sh) for a in concat_in]
    outs = zmaker()
    # warm-up + establish donation chain
    outs = sharded(*dev_in, *outs)
    for o in outs:
        o.block_until_ready()
    t0 = time.perf_counter()
    for _ in range(iters):
        outs = sharded(*dev_in, *outs)
    for o in outs:
        o.block_until_ready()
    t1 = time.perf_counter()
    return (t1 - t0) * 1e9 / iters


def benchmark(inputs, iters=8, n_lo=8, n_hi=32, rounds=3):
    """Device time per kernel execution: bake a device-side For_i loop of N
    iterations around the pipeline into the NEFF; the slope between two N
    values cancels all per-dispatch overhead (axon RTT, NEFF load). Median
    over interleaved rounds rejects transient device slowdowns."""
    import statistics
    prev = os.environ.get("KERNEL_LOOP", "0")
    slopes = []
    try:
        for _ in range(rounds):
            os.environ["KERNEL_LOOP"] = str(n_lo)
            tlo = _chain_time(inputs, 1, iters)
            os.environ["KERNEL_LOOP"] = str(n_hi)
            thi = _chain_time(inputs, 1, iters)
            slopes.append((thi - tlo) / (n_hi - n_lo))
    finally:
        os.environ["KERNEL_LOOP"] = prev
    return statistics.median(slopes)
